# revision 38
# baseline (speedup 1.0000x reference)
"""Trainium2 Bass kernel for AdDiffSortLoss (v2).

Reference computation (per batch row, n=8):
  rank_r      = # { j : labels[j] > labels[r] }          (descending rank)
  G[r, c]     = (rank_r == c)                            (one-hot GT permutation^T)
  x           = -(pred - rank_ema[rank])                 (rank_ema == 0 in practice)
  P           = odd-even differentiable sort network on x (8 layers, Cauchy CDF)
  loss        = -mean( G*clip(log P,-100) + (1-G)*clip(log1p(-P),-100) )

Decomposition on device (clips never bind for this data regime):
  sum = SUM_all ln(1-P) + SUM_r [ ln(P[r,rank_r]) - ln(1-P[r,rank_r]) ]
  loss = -sum / (B*64)

v2 design notes (cost-model simmed + A/B-tested on hardware):
 - The ACT Arctan table covers the full input range (verified on HW, err
   <= 4e-7 up to |x|~4e3), so alpha = 0.5 + atan(delta)/pi is computed
   directly: DVE does only delta/tv/x-updates (4 bf16 2x TT per layer);
   ACT produces at, al (contiguous) and the interleaved al2 pairs.
 - Value recurrence keeps even/odd lanes in separate planes so every
   phase-1 TT has step-1 bf16 APs (2x mode).
 - Comparisons run in bf16 so both is_gt operands can present step-1
   innermost dims (2x): ACT builds a pair-duplicated label copy
   rep2[f,j,:]=[L_j,L_j].  bf16 label ties perturb ~0.1% of rows'
   ground-truth ranks; measured loss impact is < 1e-4 relative.
 - CT is built transposed (CT[f,j,r]) so the rank tree keeps an 8-wide
   step-1 innermost dim at every level.
 - Layers 0+1 of the permutation recurrence are analytic: after layer 1
   every nonzero P^T entry is (L0 factor) * (L1 factor).  ACT builds the
   factor vectors AVEC/BVEC from the saved layer-0/1 arctans; the DVE
   applies two 2x multiplies.  Mixing proper starts at layer 2.
 - Only the P^T cells actually read before being written are zeroed
   (24+12 of 2x64, via strided u16 AND-0 TensorScalar 4x ops; Memset
   runs ~1x and is avoided everywhere).
 - Mixing layers 1-7 operate on true column-support windows (w=4/6/8)
   instead of always w=8, split into uniform-stride groups; the al*d
   multiply reads interleaved [al,al] bf16 pairs to stay in 2x mode.
 - Emission is software-pipelined: phase-1 of chunk c+1 and the
   label-side head of chunk c interleave with mixing so the serial
   delta->atan->al->tv chain hides under independent DVE work.

Hardware A/B findings baked in (each of these looked fine or better in
the cost-model sim but regressed badly on silicon):
 - gpsimd (Pool) memset/tensor ops: Pool shares the DVE SBUF port;
   offloading zeroing to Pool cost ~+100us.  TensorTensor on Pool does
   not even compile (ISA engine check).
 - In-place DVE tensor_tensor (out aliasing an input AP): ~2x slowdown.
 - ACT activation with a freshly-written [P,1] tensor bias (dynamic
   accumulator combine): ~+100us.  Constant [P,1] bias tiles are fine.

Sharding: pure data parallel over the batch across 8 NeuronCores; each core
reduces its shard to a [128,1] per-partition partial that the host sums.
"""

import math
import numpy as np

import concourse.bass as bass
import concourse.bacc as bacc
import concourse.tile as tile
from concourse import mybir
from concourse.bass_utils import run_bass_kernel_spmd

import ml_dtypes

F32 = mybir.dt.float32
U32 = mybir.dt.uint32
U16 = mybir.dt.uint16
BF16 = mybir.dt.bfloat16

N = 8                  # row width
N_CORES = 8
BATCH = 262144
ROWS_PER_CORE = BATCH // N_CORES   # 32768
P = 128                # partitions
RPP = ROWS_PER_CORE // P           # rows per partition (256)

A = mybir.AluOpType
AF = mybir.ActivationFunctionType

# Mixing-layer plans for layers 1..7.  Each layer: list of groups
# (a_off, b_off, pair_stride, npair, w, pair_subset_start, pair_subset_step)
# where offsets index into the [F, 8(col), 8(row)] P^T tile (col*8 + row).
# Window = true union of the two columns' supports (grows ~1 row/layer).
MIX_PLAN = {
    2: [(0, 8, 52, 2, 4, 0, 3),      # pairs (0,1),(6,7)
        (16, 24, 18, 2, 6, 1, 1)],   # pairs (2,3),(4,5)
    3: [(8, 16, 34, 2, 6, 0, 2),     # pairs (1,2),(5,6)
        (24, 32, 8, 1, 8, 1, 1)],    # pair  (3,4)
    4: [(0, 8, 50, 2, 6, 0, 3),      # pairs (0,1),(6,7)
        (16, 24, 16, 2, 8, 1, 1)],   # pairs (2,3),(4,5)
    5: [(8, 16, 16, 3, 8, 0, 1)],
    6: [(0, 8, 16, 4, 8, 0, 1)],
    7: [(8, 16, 16, 3, 8, 0, 1)],
}
# passthrough copy widths for cols 0/7 after odd layers (their support)
PASS_W = {3: 4, 5: 6, 7: 8}
PRE_DRIVE = 0  # phase1(c+1) blocks emitted before zipping with mix(c).
# 3 would eliminate ACT arctan<->ln table-set thrash in the static schedule
# but measures ~60us SLOWER on silicon (A/B tested); keep 0.



def build_nc(rows_per_core=ROWS_PER_CORE, chunk_rows=128, mix_bf16=True,
             repeats=1):
    """Build the single-core SPMD Bass graph."""
    rpp = rows_per_core // P
    assert rpp * P == rows_per_core
    F = chunk_rows
    n_chunks = rpp // F
    assert n_chunks * F == rpp

    nc = bacc.Bacc("TRN2")

    pred_h = nc.declare_dram_parameter("pred", [rows_per_core, N], F32, isOutput=False)
    lab_h = nc.declare_dram_parameter("labels", [rows_per_core, N], F32, isOutput=False)
    # iota_cr[p, c*8+r] = c (replicated across partitions) -- for GT construction
    iota_h = nc.declare_dram_parameter("iota_cr", [P, N * N], BF16, isOutput=False)
    out_h = nc.declare_dram_parameter("out", [P, 6], F32, isOutput=True)

    predv = pred_h[:].rearrange("(p f) n -> p f n", p=P)   # [128, rpp, 8]
    labv = lab_h[:].rearrange("(p f) n -> p f n", p=P)

    with tile.TileContext(nc) as tc:
        with (
            tc.tile_pool(name="io", bufs=1) as io,
            tc.tile_pool(name="p1", bufs=1) as p1p,
            tc.tile_pool(name="al", bufs=1) as alp,
            tc.tile_pool(name="pt", bufs=1) as ptp,
            tc.tile_pool(name="rk", bufs=1) as rk,
            tc.tile_pool(name="mt", bufs=1) as mt,
            tc.tile_pool(name="acc", bufs=2) as accp,
            tc.tile_pool(name="singles", bufs=1) as singles,
        ):
            # ---------------- constants ----------------
            iota_t = singles.tile([P, N * N], BF16, tag="iota")
            nc.sync.dma_start(out=iota_t, in_=iota_h[:])
            total_t = singles.tile([P, 1], F32, tag="total")
            # total = iota[:, :1] & 0 — zero-init that also consumes the iota
            # DMA on the DVE so later wide ops reading iota_t need no extra
            # sync-wait slot.
            nc.vector.tensor_scalar(
                total_t.bitcast(U32), iota_t[:, 0:2].bitcast(U32),
                0, None, A.bitwise_and,
            )
            half_t = singles.tile([P, 1], F32, tag="half")
            nc.vector.memset(half_t, 0.5)
            # warm the arctan table set while the input DMAs are in flight
            warm_t = singles.tile([P, 1], BF16, tag="warm")
            nc.scalar.activation(warm_t, half_t, AF.Arctan)

            outacc_t = singles.tile([P, 3 * n_chunks], F32, tag="outacc")
            for _ in range(repeats):

                # al2 group tiles, per (chunk, layer, group)
                al2s = {}
                at01s = {}

                # ---------- phase 1 generator (per chunk) ----------
                def phase1(c):
                    sl = slice(c * F, (c + 1) * F)
                    pred_t = io.tile([P, F, N], F32, tag="pred")
                    nc.sync.dma_start(out=pred_t, in_=predv[:, sl, :])
                    xe = p1p.tile([P, F, 4], BF16, tag=f"xe_a{c}")
                    xo = p1p.tile([P, F, 4], BF16, tag=f"xo_a{c}")
                    xe2 = p1p.tile([P, F, 4], BF16, tag=f"xe_b{c}")
                    xo2 = p1p.tile([P, F, 4], BF16, tag=f"xo_b{c}")
                    pv = pred_t.rearrange("p f (a b) -> p f a b", b=2)
                    if c == 0:   # DVE is idle at startup; keep ACT free
                        nc.vector.tensor_scalar(
                            xe, pv[:, :, :, 0], -10.0, None, A.mult)
                        nc.vector.tensor_scalar(
                            xo, pv[:, :, :, 1], -10.0, None, A.mult)
                    else:        # DVE is busy mixing; ACT has slack
                        nc.scalar.activation(xe, pv[:, :, :, 0], AF.Identity,
                                             scale=-10.0)
                        nc.scalar.activation(xo, pv[:, :, :, 1], AF.Identity,
                                             scale=-10.0)
                    yield
                    for layer in range(N):
                        odd = layer % 2
                        npair = 3 if odd else 4
                        if odd:
                            a_ap = xo[:, :, 0:3]
                            b_ap = xe[:, :, 1:4]
                        else:
                            a_ap = xe[:, :, 0:4]
                            b_ap = xo[:, :, 0:4]
                        dl = p1p.tile([P, F, npair], BF16, tag=f"dl{c}_{odd}")  # per-parity reuse ok: consumed within layer
                        nc.vector.tensor_tensor(dl, b_ap, a_ap, A.subtract)
                        at = p1p.tile(
                            [P, F, npair], BF16,
                            tag=f"at{c}_L{layer}" if layer < 2 else f"at{c}_{odd}",
                        )
                        nc.scalar.activation(at, dl, AF.Arctan)
                        if layer < N - 1:
                            al_t = p1p.tile([P, F, npair], BF16,
                                            tag=f"alt{c}_{odd}")
                            nc.scalar.activation(
                                al_t, at, AF.Identity, scale=1.0 / math.pi,
                                bias=half_t,
                            )
                        # interleaved al2 pair tiles per mixing group
                        if layer < 2:
                            # layers 0+1 are applied analytically (composite
                            # products of L0/L1 alphas) -- keep the at tiles
                            at01s[(c, layer)] = at
                            groups = []
                        else:
                            groups = [(g[5], g[6], g[3]) for g in MIX_PLAN[layer]]
                        for gi, (p0, pstep, gnp) in enumerate(groups):
                            g_t = alp.tile([P, F, gnp, 2], BF16,
                                           tag=f"al2_{c}_{layer}_{gi}")
                            at_g = bass.AP(
                                tensor=at.tensor, offset=at.offset + p0,
                                ap=[at.ap[0], [npair, F], [pstep, gnp]],
                            )
                            for half in range(2):
                                out_g = bass.AP(
                                    tensor=g_t.tensor, offset=g_t.offset + half,
                                    ap=[g_t.ap[0], [gnp * 2, F], [2, gnp]],
                                )
                                nc.scalar.activation(
                                    out_g, at_g, AF.Identity,
                                    scale=1.0 / math.pi, bias=half_t,
                                )
                            al2s[(c, layer, gi)] = g_t
                        if layer == N - 1:
                            # the sorted values themselves are never used --
                            # only the alphas feed phase 2.  Last layer needs
                            # no value update.
                            yield
                            continue
                        tv = p1p.tile([P, F, npair], BF16, tag=f"tv{c}_{odd}")
                        nc.vector.tensor_tensor(tv, al_t, dl, A.mult)
                        if odd:
                            nc.vector.tensor_tensor(
                                xo2[:, :, 0:3], b_ap, tv, A.subtract
                            )
                            nc.vector.tensor_tensor(
                                xe2[:, :, 1:4], a_ap, tv, A.add
                            )
                            nc.scalar.activation(
                                xe2[:, :, 0:1], xe[:, :, 0:1], AF.Identity
                            )
                            nc.scalar.activation(
                                xo2[:, :, 3:4], xo[:, :, 3:4], AF.Identity
                            )
                        elif layer == N - 2:
                            # layer 7 reads only xo'[0:3] / xe'[1:4]
                            nc.vector.tensor_tensor(
                                xe2[:, :, 1:4], xo[:, :, 1:4],
                                tv[:, :, 1:4], A.subtract,
                            )
                            nc.vector.tensor_tensor(
                                xo2[:, :, 0:3], xe[:, :, 0:3],
                                tv[:, :, 0:3], A.add,
                            )
                        else:
                            nc.vector.tensor_tensor(
                                xe2[:, :, 0:4], b_ap, tv, A.subtract
                            )
                            nc.vector.tensor_tensor(
                                xo2[:, :, 0:4], a_ap, tv, A.add
                            )
                        xe, xe2 = xe2, xe
                        xo, xo2 = xo2, xo
                        yield

                # ---------- phase 2 head (labels side) ----------
                def head(c):
                    sl = slice(c * F, (c + 1) * F)
                    lab_k = io.tile([P, F, N], F32, tag="lab")
                    nc.sync.dma_start(out=lab_k, in_=labv[:, sl, :])
                    # CT[f, j, r] = (L[j] > L[r])
                    CT = rk.tile([P, F, N, N], BF16, tag="CT")
                    # bf16 compares reach DVE 2x mode: the r-broadcast
                    # side needs an innermost step-1 dim, so ACT builds a
                    # pair-duplicated label copy rep2[f,j,:]=[L_j,L_j].
                    lab_b = io.tile([P, F, N], BF16, tag="lab_b")
                    nc.scalar.activation(lab_b, lab_k, AF.Identity)
                    rep2 = io.tile([P, F, N, 2], BF16, tag="rep2")
                    for half in range(2):
                        dst = bass.AP(
                            tensor=rep2.tensor, offset=rep2.offset + half,
                            ap=[rep2.ap[0], [2 * N, F], [2, N]],
                        )
                        nc.scalar.activation(dst, lab_b, AF.Identity)
                    in_lj = bass.AP(
                        tensor=rep2.tensor, offset=rep2.offset,
                        ap=[rep2.ap[0], [2, F * N], [0, N // 2], [1, 2]],
                    )
                    in_lr = bass.AP(
                        tensor=lab_b.tensor, offset=lab_b.offset,
                        ap=[lab_b.ap[0], [N, F], [0, N], [1, N]],
                    )
                    nc.vector.tensor_tensor(CT, in_lj, in_lr, A.is_gt)
                    yield
                    cs1 = rk.tile([P, F, 4, N], BF16, tag="cs1")
                    nc.vector.tensor_tensor(
                        cs1, CT[:, :, 0:4, :], CT[:, :, 4:8, :], A.add
                    )
                    yield
                    cs2 = rk.tile([P, F, 2, N], BF16, tag="cs2")
                    nc.vector.tensor_tensor(
                        cs2, cs1[:, :, 0:2, :], cs1[:, :, 2:4, :], A.add
                    )
                    rank_t = rk.tile([P, F, N], BF16, tag="rank")
                    nc.vector.tensor_tensor(
                        rank_t, cs2[:, :, 0, :], cs2[:, :, 1, :], A.add
                    )
                    yield
                    # GT[f, c, r] = (rank[f, r] == c)   (both inputs step-1)
                    GT = rk.tile([P, F, N, N], BF16, tag="GT")
                    in_rank = rank_t.unsqueeze(2).broadcast_to([P, F, N, N])
                    in_iota = (
                        iota_t.rearrange("p (c r) -> p c r", c=N)
                        .unsqueeze(1)
                        .broadcast_to([P, F, N, N])
                    )
                    nc.vector.tensor_tensor(GT, in_rank, in_iota, A.is_equal)
                    yield
                    # zero both P^T buffers (u16 AND 0: 4x + NaN-proof)
                    pt_a = ptp.tile([P, F, N, N], BF16, tag="pt_a")
                    pt_b = ptp.tile([P, F, N, N], BF16, tag="pt_b")
                    # zero exactly the cells each buffer exposes to a read
                    # before writing them.  In flat (col*8+row) offset space
                    # the must-zero cells form uniform stride-18 runs:
                    # pt_a: {2-7, 20-25, 38-43, 56-61}; pt_b: {12-15, 30-33,
                    # 48-51} -- one 4x u16 AND-0 op per buffer.
                    for pt, (off, nrun, w) in (
                        (pt_a, (2, 4, 6)),
                        (pt_b, (12, 3, 4)),
                    ):
                        z = bass.AP(
                            tensor=pt.tensor, offset=pt.offset + off,
                            ap=[pt.ap[0], [N * N, F], [18, nrun], [1, w]],
                        )
                        nc.vector.tensor_scalar(
                            z.bitcast(U16), z.bitcast(U16), 0, None,
                            A.bitwise_and,
                        )
                        yield
                    head.out[c] = (GT, pt_a, pt_b)
                    yield
                head.out = {}

                def col_ap(pt, off, cstride, npair, w):
                    return bass.AP(
                        tensor=pt.tensor, offset=pt.offset + off,
                        ap=[pt.ap[0], [N * N, F], [cstride, npair], [1, w]],
                    )

                # ---------- phase 2 mixing (layers 0+1 analytic) ----
                def mix_l01(c, pt_a):
                    # After layer 1 every nonzero P^T entry is a product of
                    # one L0 factor and one L1 factor:
                    #   col 2j+1 rows 2j..2j+3 = AVEC[j,:] * [b,b,1-b,1-b]
                    #   col 2j+2 rows 2j..2j+3 = AVEC[j,:] * [1-b,1-b,b,b]
                    # with AVEC[j] = [om0_j, al0_j, al0_{j+1}, om0_{j+1}],
                    # b = alpha of L1 pair j.  ACT builds the factor vectors;
                    # the DVE does just two 2x multiplies.  Cols 0/7 are pure
                    # L0 values written by ACT.
                    at0 = at01s[(c, 0)]
                    at1 = at01s[(c, 1)]
                    avec = mt.tile([P, F, 3, 4], BF16, tag="avec")
                    for k, joff, sgn in ((0, 0, -1.0), (1, 0, 1.0),
                                         (2, 1, 1.0), (3, 1, -1.0)):
                        dst = bass.AP(
                            tensor=avec.tensor, offset=avec.offset + k,
                            ap=[avec.ap[0], [12, F], [4, 3]],
                        )
                        src = bass.AP(
                            tensor=at0.tensor, offset=at0.offset + joff,
                            ap=[at0.ap[0], [4, F], [1, 3]],
                        )
                        nc.scalar.activation(
                            dst, src, AF.Identity, scale=sgn / math.pi,
                            bias=half_t,
                        )
                    bv1 = mt.tile([P, F, 3, 4], BF16, tag="bv1")
                    bv2 = mt.tile([P, F, 3, 4], BF16, tag="bv2")
                    for k in range(4):
                        sg1 = 1.0 if k < 2 else -1.0
                        for bv, sg in ((bv1, sg1), (bv2, -sg1)):
                            dst = bass.AP(
                                tensor=bv.tensor, offset=bv.offset + k,
                                ap=[bv.ap[0], [12, F], [4, 3]],
                            )
                            nc.scalar.activation(
                                dst, at1, AF.Identity, scale=sg / math.pi,
                                bias=half_t,
                            )
                    nc.vector.tensor_tensor(
                        col_ap(pt_a, 8, 18, 3, 4), avec, bv1, A.mult
                    )
                    nc.vector.tensor_tensor(
                        col_ap(pt_a, 16, 18, 3, 4), avec, bv2, A.mult
                    )
                    # col0 rows 0,1 = [al0_0, om0_0]; col7 rows 6,7 = [om0_3, al0_3]
                    for off, joff, sgn in ((0, 0, 1.0), (1, 0, -1.0),
                                           (62, 3, -1.0), (63, 3, 1.0)):
                        dst = bass.AP(
                            tensor=pt_a.tensor, offset=pt_a.offset + off,
                            ap=[pt_a.ap[0], [N * N, F]],
                        )
                        src = bass.AP(
                            tensor=at0.tensor, offset=at0.offset + joff,
                            ap=[at0.ap[0], [4, F]],
                        )
                        nc.scalar.activation(
                            dst, src, AF.Identity, scale=sgn / math.pi,
                            bias=half_t,
                        )

                def mix_layers(c, pt_a, pt_b):
                    pt_cur, pt_nxt = pt_a, pt_b
                    for layer in range(2, N):
                        for gi, (a_off, b_off, cstride, gnp, w, _, _) in \
                                enumerate(MIX_PLAN[layer]):
                            al2g = al2s[(c, layer, gi)]
                            A_ap = col_ap(pt_cur, a_off, cstride, gnp, w)
                            B_ap = col_ap(pt_cur, b_off, cstride, gnp, w)
                            d = mt.tile([P, F, gnp, w], BF16, tag=f"d{w}_{gnp}")
                            nc.vector.tensor_tensor(d, A_ap, B_ap, A.subtract)
                            al2_v = bass.AP(
                                tensor=al2g.tensor, offset=al2g.offset,
                                ap=[al2g.ap[0], [2, F * gnp], [0, w // 2],
                                    [1, 2]],
                            )
                            d_v = bass.AP(
                                tensor=d.tensor, offset=d.offset,
                                ap=[d.ap[0], [w, F * gnp], [2, w // 2], [1, 2]],
                            )
                            t = mt.tile([P, F, gnp, w], BF16, tag=f"t{w}_{gnp}")
                            t_v = bass.AP(
                                tensor=t.tensor, offset=t.offset,
                                ap=[t.ap[0], [w, F * gnp], [2, w // 2], [1, 2]],
                            )
                            nc.vector.tensor_tensor(t_v, al2_v, d_v, A.mult)
                            nc.vector.tensor_tensor(
                                col_ap(pt_nxt, a_off, cstride, gnp, w),
                                B_ap, t, A.add,
                            )
                            nc.vector.tensor_tensor(
                                col_ap(pt_nxt, b_off, cstride, gnp, w),
                                A_ap, t, A.subtract,
                            )
                        if layer % 2 == 1:  # passthrough cols 0 and 7
                            w = PASS_W[layer]
                            nc.vector.tensor_copy(
                                col_ap(pt_nxt, 0, 56 + N - w, 2, w),
                                col_ap(pt_cur, 0, 56 + N - w, 2, w),
                            )
                        pt_cur, pt_nxt = pt_nxt, pt_cur
                        yield
                    mix_layers.out[c] = (pt_cur, pt_nxt)
                mix_layers.out = {}

                # ---------- phase 2 tail (pick + BCE) ----------
                def tail(c):
                    GT, _, _ = head.out[c]
                    pt_cur, pt_nxt = mix_layers.out[c]
                    Q = rk.tile([P, F, N, N], BF16, tag="CT")  # safe: delayed head
                    nc.vector.tensor_tensor(Q, GT, pt_cur, A.mult)
                    yield
                    qs1 = rk.tile([P, F, 4, N], BF16, tag="cs1")
                    nc.vector.tensor_tensor(
                        qs1, Q[:, :, 0:4, :], Q[:, :, 4:8, :], A.add
                    )
                    yield
                    qs2 = rk.tile([P, F, 2, N], BF16, tag="cs2")
                    nc.vector.tensor_tensor(
                        qs2, qs1[:, :, 0:2, :], qs1[:, :, 2:4, :], A.add
                    )
                    sel = rk.tile([P, F, N], BF16, tag="rank")
                    nc.vector.tensor_tensor(
                        sel, qs2[:, :, 0, :], qs2[:, :, 1, :], A.add
                    )
                    yield
                    acc1 = outacc_t[:, 3 * c:3 * c + 1]
                    ln_scr = pt_nxt  # idle ping-pong buffer after 8 layers
                    nc.scalar.activation(
                        ln_scr.rearrange("p a b c -> p (a b c)"),
                        pt_cur.rearrange("p a b c -> p (a b c)"),
                        AF.Ln, scale=-1.0, bias=1.0, accum_out=acc1,
                    )
                    acc2 = outacc_t[:, 3 * c + 1:3 * c + 2]
                    sel_scr = rk.tile([P, F, N], BF16, tag="sel_scr")
                    nc.scalar.activation(
                        sel_scr.rearrange("p a b -> p (a b)"),
                        sel.rearrange("p a b -> p (a b)"),
                        AF.Ln, accum_out=acc2,
                    )
                    acc3 = outacc_t[:, 3 * c + 2:3 * c + 3]
                    sel_scr2 = rk.tile([P, F, N], BF16, tag="sel_scr")
                    nc.scalar.activation(
                        sel_scr2.rearrange("p a b -> p (a b)"),
                        sel.rearrange("p a b -> p (a b)"),
                        AF.Ln, scale=-1.0, bias=1.0, accum_out=acc3,
                    )
                    yield

                def zip_emit(*gens):
                    alive = list(gens)
                    while alive:
                        for g in list(alive):
                            try:
                                next(g)
                            except StopIteration:
                                alive.remove(g)

                # ---------- pipelined emission ----------
                for c in range(n_chunks):
                    if c == 0:
                        zip_emit(phase1(0), head(0))
                    GT_c, pt_a, pt_b = head.out[c]
                    mix_l01(c, pt_a)
                    gens = [mix_layers(c, pt_a, pt_b)]
                    if c + 1 < n_chunks:
                        p1n = phase1(c + 1)
                        for _ in range(PRE_DRIVE):
                            next(p1n)
                        gens.append(p1n)
                    zip_emit(*gens)
                    tg = tail(c)
                    next(tg); next(tg); next(tg)  # Q/qs trees before head(c+1)
                    gens = [tg]
                    if c + 1 < n_chunks:
                        gens.append(head(c + 1))
                    zip_emit(*gens)

            nc.gpsimd.dma_start(out=out_h[:], in_=outacc_t)

    nc.compile()
    return nc


_NC_CACHE = {}


def _get_nc(rows_per_core, chunk_rows=128, mix_bf16=True, repeats=1):
    key = (rows_per_core, chunk_rows, mix_bf16, repeats)
    if key not in _NC_CACHE:
        _NC_CACHE[key] = build_nc(rows_per_core, chunk_rows, mix_bf16, repeats)
    return _NC_CACHE[key]


def _iota_const(mix_bf16=True):
    row = np.repeat(np.arange(N), N).astype(ml_dtypes.bfloat16)  # iota_cr[c*8+r] = c
    return np.ascontiguousarray(np.broadcast_to(row, (P, N * N)))


def run_on_device(pred, labels, chunk_rows=128, mix_bf16=True, trace=False):
    """pred/labels: full [BATCH, 8] f32 (already ema-shifted). Returns
    (loss_scalar_f32, BassKernelResults)."""
    rows = pred.shape[0] // N_CORES
    nc = _get_nc(rows, chunk_rows, mix_bf16)
    iota = _iota_const(mix_bf16)
    in_maps = [
        {
            "pred": np.ascontiguousarray(pred[i * rows:(i + 1) * rows]),
            "labels": np.ascontiguousarray(labels[i * rows:(i + 1) * rows]),
            "iota_cr": iota,
        }
        for i in range(N_CORES)
    ]
    res = run_bass_kernel_spmd(nc, in_maps, list(range(N_CORES)), trace=trace)
    total = np.float64(0.0)
    for r in res.results:
        o = np.asarray(r["out"], dtype=np.float64)
        total += o[:, 0::3].sum() + o[:, 1::3].sum() - o[:, 2::3].sum()
    loss = -total / (pred.shape[0] * N * N)
    return np.float32(loss), res


def kernel(pred_scores, labels, rank_ema):
    pred = np.asarray(pred_scores, dtype=np.float32)
    lab = np.asarray(labels, dtype=np.float32)
    ema = np.asarray(rank_ema, dtype=np.float32)
    if np.any(ema != 0.0):
        # General path: fold the (tiny, data-independent-size) EMA shift on
        # host; the device graph is unchanged. rank_true = rank of each label.
        order = np.argsort(-lab, axis=-1, kind="stable")
        rank_true = np.argsort(order, axis=-1, kind="stable")
        pred = (pred - ema[rank_true]).astype(np.float32)
    loss, _ = run_on_device(pred, lab)
    return np.array(loss, dtype=np.float32)
